# revision 6
# baseline (speedup 1.0000x reference)
"""Category-specific linear: out[b] = x[b] @ weight[cat[b]] + bias[cat[b]].

Full shapes: x [32, 512, 1024] f32, category_ids [32] int, weight
[64, 1024, 1024] f32, bias [64, 1024] f32 -> out [32, 512, 1024] f32.

Strategy: data-parallel over batch across 8 NeuronCores (4 batches/core).
All device-side numerics run in fp16: the host gathers per-batch weights,
pre-transposes x to [K, L], and casts both to fp16 (halving the HBM
stream vs f32); the device writes fp16 output and the host casts up,
restores the layout, and adds the bias. With a 16 MB per-core HBM stream
the kernel is PE-bound (~55 us of matmul at the 2.4 GHz fast clock), so
everything serves the matmul issue rate:

- x is the stationary operand (lhsT = xt[k, lt] tile [128K x 128L]) and
  w the moving one (rhs = w[k] in two [128K x 512N] chunks), so each
  stationary tile serves 2 consecutive matmuls and the PE array's
  weight-swap drain is paid once per pair. PSUM holds 8 bank tiles
  [128L x 512N] per batch; measured issue rate 216 ns/matmul.
- A run of small (N=128) dummy matmuls on garbage SBUF warms the PE HAM
  clock gate during the framework preamble + first-chunk DMA fill, so
  real matmuls start at the fast clock with fine-grained (~107 ns cold)
  handoff to the first data-dependent matmul.
- Inputs stream as uniform chunks (batch, k-tile) in consumption order,
  balanced across the two HWDGE rings (even k on sync, odd k on ACT,
  ~6 MB each) with every input DMA issued up front and no waits in the
  issue stream, so the queues run at combined HBM rate end-to-end. The
  host packs xt|w for each chunk contiguously (p-major, 3 KB
  per-partition lines), so a chunk is ONE 384 KB DMA. The very first
  k-chunk is split across BOTH rings (xt + w-half1 on sync, w-half0 on
  ACT) to minimize time-to-first-real-matmul. Batch 0-2 outputs ride
  the otherwise-idle GPSIMD SWDGE queue so they never contend with
  input issue order.
- PSUM eviction of each row tile is split column-wise between the DVE
  (vector) and ACT (scalar) engines - two parallel copy-with-cast ops -
  so the next batch's matmuls never stall on bank reuse.
- Batches 0-2 interleave their last two k-rounds per tile so every
  tile's final matmul (and its eviction) lands well before the batch
  boundary. The LAST batch instead runs a per-tile staircase over its
  last four k-rounds: row tiles complete (and stream out, one 256 KB
  DMA per tile alternating rings) one at a time, so the ~1 MB of
  final-batch output drains progressively instead of bunching after the
  last matmul; only the last row tile's 256 KB remains for the tail.
"""

from contextlib import ExitStack

import numpy as np

import concourse.bass as bass
import concourse.mybir as mybir
from concourse.bass_utils import run_bass_kernel_spmd

# Per-core problem shape
B = 4           # batches per core
L = 512         # rows (seq positions) per batch
K = 1024        # contraction dim
N = 1024        # output dim
KT = K // 128   # 8 k-tiles
LT = L // 128   # 4 l-tiles (psum row tiles per batch, 2 banks each)
NWARM = 24      # small dummy matmuls to warm the PE clock before data

F32 = mybir.dt.float32
F16 = mybir.dt.float16
NP_DT = np.float16

CH = L + N       # 1536: packed chunk width (xt k-tile | w k-tile)
BBUF = KT * CH   # 12288 fp16 per partition per batch
OBUF = LT * N    # 4096


def build_program() -> bass.Bass:
    nc = bass.Bass()

    # p-major packing: element [b, p, k*CH + f] is k-row k*128+p,
    # f in [0,512) -> xt col f;  f in [512,1536) -> w col f-512
    in_d = nc.declare_dram_parameter("inp", [B, 128, BBUF], F16, isOutput=False)
    out_d = nc.declare_dram_parameter("out", [B, 128, OBUF], F16, isOutput=True)

    with ExitStack() as ctx:
        # all 4 batches resident: 96 KB/part inputs + 16 KB out
        in_sb = ctx.enter_context(nc.sbuf_tensor([128, B * BBUF], F16))
        out_sb = ctx.enter_context(nc.sbuf_tensor([128, 2 * OBUF], F16))
        psum = ctx.enter_context(nc.psum_tensor([128, 8 * 512], F32))  # 8 banks
        s_chunk = [ctx.enter_context(nc.semaphore(f"s_c{k}")) for k in range(KT)]
        s_o = [ctx.enter_context(nc.semaphore(f"s_o{b}")) for b in range(B)]
        s_c0a = ctx.enter_context(nc.semaphore("s_c0a"))  # b0 k0 xt
        s_c0w = ctx.enter_context(nc.semaphore("s_c0w"))  # b0 k0 w half0
        s_c0b = ctx.enter_context(nc.semaphore("s_c0b"))  # b0 k0 w half1
        s_mm = ctx.enter_context(nc.semaphore("s_mm"))
        s_cpv = ctx.enter_context(nc.semaphore("s_cpv"))
        s_cps = ctx.enter_context(nc.semaphore("s_cps"))
        block = ctx.enter_context(nc.Block())

        def xt_tile(b, k, lt):
            base = b * BBUF + k * CH + lt * 128
            return in_sb[:, base : base + 128]

        def w_half(b, k, nh):
            base = b * BBUF + k * CH + L + nh * 512
            return in_sb[:, base : base + 512]

        def load_chunk(eng, b, k):
            eng.dma_start(
                out=in_sb[:, b * BBUF + k * CH : b * BBUF + (k + 1) * CH],
                in_=in_d[b, :, k * CH : (k + 1) * CH],
            ).then_inc(s_chunk[k], 16)

        # s_mm increments: two per row tile (k7-nh0, k7-nh1), in tile-
        # completion order; batch 3's staircase completes tiles one at a
        # time.
        MM_B3 = 3 * 2 * LT  # 24 incs before batch 3

        @block.sync
        def _(sync):
            # Ring balance: this ring carries b0's k0-xt and k0-w-half1
            # pieces plus ALL even k-chunks; the ACT ring carries k0-w-half0
            # plus ALL odd k-chunks (~6 MB each). Every input DMA is issued
            # up front with no waits in between, so the two HWDGE queues
            # stream the whole 12 MB input in consumption order at combined
            # HBM rate with nothing blocking the issue stream. Separate
            # semaphores for the three k0 pieces: DMA-completion increments
            # arrive per SDMA slot, so a shared counter could hit 16 from a
            # mix of pieces while one is incomplete.
            sync.dma_start(
                out=in_sb[:, 0:L], in_=in_d[0, :, 0:L]
            ).then_inc(s_c0a, 16)
            sync.dma_start(
                out=in_sb[:, L + 512 : CH], in_=in_d[0, :, L + 512 : CH]
            ).then_inc(s_c0b, 16)
            for b in range(B):
                for k in range(0 if b else 2, KT, 2):
                    load_chunk(sync, b, k)
            # batch-3 staircase tail on this ring: row tile lt2, then lt3's
            # nh0 half (the nh1 half rides the scalar ring).
            ob = (B - 1) % 2 * OBUF
            sync.wait_ge(s_cpv, 3 * LT + 3)
            sync.wait_ge(s_cps, 3 * LT + 3)
            sync.dma_start(
                out=out_d[B - 1, :, 2 * N : 3 * N],
                in_=out_sb[:, ob + 2 * N : ob + 3 * N],
            ).then_inc(s_o[B - 1], 16)
            sync.wait_ge(s_cpv, 3 * LT + 4)
            sync.dma_start(
                out=out_d[B - 1, :, 3 * N : 3 * N + 512],
                in_=out_sb[:, ob + 3 * N : ob + 3 * N + 512],
            ).then_inc(s_o[B - 1], 16)
            sync.drain()

        @block.scalar
        def _(scalar):
            # b0 k0 w-half0 first (gates the first real matmuls), then all
            # odd k-chunks for every batch, issued back-to-back.
            scalar.dma_start(
                out=in_sb[:, L : L + 512], in_=in_d[0, :, L : L + 512]
            ).then_inc(s_c0w, 16)
            for b in range(B):
                for k in range(1, KT, 2):
                    load_chunk(scalar, b, k)
            # evictions (n-half 1 of each row tile); output DMAs for
            # batches 0-2 ride the otherwise-idle GPSIMD SWDGE ring so they
            # never queue behind input chunks here.
            for b in range(B - 1):
                obuf = b % 2
                if b >= 2:
                    scalar.wait_ge(s_o[b - 2], 32)
                for lt in range(LT):
                    scalar.wait_ge(s_mm, b * 2 * LT + lt * 2 + 2)
                    nc.scalar.copy(
                        out=out_sb[
                            :,
                            obuf * OBUF + lt * N + 512 : obuf * OBUF + (lt + 1) * N,
                        ],
                        in_=psum[:, (lt * 2 + 1) * 512 : (lt * 2 + 2) * 512],
                    ).then_inc(s_cps, 1)
            # batch 3 staircase: evict each tile's nh1 as it completes;
            # per-tile outputs leave immediately (lt0/lt1 on gpsimd, lt2 +
            # lt3-nh0 on sync, the final lt3-nh1 piece here).
            b, obuf = B - 1, (B - 1) % 2
            scalar.wait_ge(s_o[b - 2], 32)
            for lt in range(LT):
                scalar.wait_ge(s_mm, MM_B3 + lt * 2 + 2)
                nc.scalar.copy(
                    out=out_sb[
                        :,
                        obuf * OBUF + lt * N + 512 : obuf * OBUF + (lt + 1) * N,
                    ],
                    in_=psum[:, (lt * 2 + 1) * 512 : (lt * 2 + 2) * 512],
                ).then_inc(s_cps, 1)
            # the tail piece: last tile's nh1 half, 128 KB
            scalar.dma_start(
                out=out_d[b, :, (LT - 1) * N + 512 : LT * N],
                in_=out_sb[
                    :,
                    obuf * OBUF + (LT - 1) * N + 512 : obuf * OBUF + LT * N,
                ],
            ).then_inc(s_o[b], 16)

        @block.gpsimd
        def _(gpsimd):
            # Output ring: batches 0-2 (two 512 KB DMAs each) plus batch
            # 3's first two staircase tiles. SWDGE shares the 16 SDMA
            # engines with the HWDGE rings but has its own queue, so
            # outputs stream concurrently with (not behind) input chunks.
            for b in range(B - 1):
                obuf = b % 2
                for c in range(2):
                    gpsimd.wait_ge(s_cpv, b * LT + 2 * c + 2)
                    gpsimd.wait_ge(s_cps, b * LT + 2 * c + 2)
                    lo, hi = c * 2 * N, (c + 1) * 2 * N
                    gpsimd.dma_start(
                        out=out_d[b, :, lo:hi],
                        in_=out_sb[:, obuf * OBUF + lo : obuf * OBUF + hi],
                    ).then_inc(s_o[b], 16)
            b, obuf = B - 1, (B - 1) % 2
            for lt in range(2):
                gpsimd.wait_ge(s_cpv, b * LT + lt + 1)
                gpsimd.wait_ge(s_cps, b * LT + lt + 1)
                gpsimd.dma_start(
                    out=out_d[b, :, lt * N : (lt + 1) * N],
                    in_=out_sb[:, obuf * OBUF + lt * N : obuf * OBUF + (lt + 1) * N],
                ).then_inc(s_o[b], 16)

        @block.tensor
        def _(tensor):
            # warm the HAM clock gate while the preamble + first DMA run;
            # small N so the handoff to the first real matmul is fine-
            # grained (~107 ns cold).
            for i in range(NWARM):
                nc.tensor.matmul(
                    psum[:, 0:128],
                    in_sb[:, 0:128],
                    in_sb[:, L : L + 128],
                    start=True,
                    stop=True,
                )

            def mm_at(b, k, lt, nh, inc=False):
                t = lt * 2 + nh
                mm = nc.tensor.matmul(
                    psum[:, t * 512 : (t + 1) * 512],
                    xt_tile(b, k, lt),
                    w_half(b, k, nh),
                    start=(k == 0),
                    stop=(k == KT - 1),
                )
                if inc:
                    mm.then_inc(s_mm, 1)

            # batch 0, k=0: nh-grouped so the nh=0 matmuls only wait for
            # the xt + w-half0 pieces of the split chunk
            tensor.wait_ge(s_c0a, 16)
            tensor.wait_ge(s_c0w, 16)
            for lt in range(LT):
                mm_at(0, 0, lt, 0)
            tensor.wait_ge(s_c0b, 16)
            for lt in range(LT):
                mm_at(0, 0, lt, 1)
            # batches 0-2: dense k-rounds, last two k-rounds interleaved
            # per tile so evictions run ahead of the batch boundary
            for b in range(B - 1):
                for k in range(1 if b == 0 else 0, KT - 2):
                    tensor.wait_ge(s_chunk[k], 16 * (b + 1 if k else b))
                    for lt in range(LT):
                        for nh in range(2):
                            if k == 0 and b > 0:
                                sem = s_cpv if nh == 0 else s_cps
                                tensor.wait_ge(sem, (b - 1) * LT + lt + 1)
                            mm_at(b, k, lt, nh)
                tensor.wait_ge(s_chunk[KT - 2], 16 * (b + 1))
                tensor.wait_ge(s_chunk[KT - 1], 16 * (b + 1))
                for lt in range(LT):
                    for k in (KT - 2, KT - 1):
                        for nh in range(2):
                            mm_at(b, k, lt, nh, inc=(k == KT - 1))
            # batch 3: k-rounds 0..3 dense, then a per-tile staircase over
            # k=4..7 so row tiles complete (and stream out) one at a time.
            b = B - 1
            for k in range(4):
                tensor.wait_ge(s_chunk[k], 16 * (b + 1 if k else b))
                for lt in range(LT):
                    for nh in range(2):
                        if k == 0:
                            sem = s_cpv if nh == 0 else s_cps
                            tensor.wait_ge(sem, (b - 1) * LT + lt + 1)
                        mm_at(b, k, lt, nh)
            for k in range(4, KT):
                tensor.wait_ge(s_chunk[k], 16 * (b + 1))
            for lt in range(LT):
                for k in range(4, KT):
                    for nh in range(2):
                        mm_at(b, k, lt, nh, inc=(k == KT - 1))

        @block.vector
        def _(vector):
            # evictions: n-half 0 of every (lt) row tile
            for b in range(B):
                obuf = b % 2
                if b >= 2:
                    vector.wait_ge(s_o[b - 2], 32)
                for lt in range(LT):
                    base = b * 2 * LT if b < B - 1 else MM_B3
                    vector.wait_ge(s_mm, base + lt * 2 + 1)
                    nc.vector.tensor_copy(
                        out=out_sb[:, obuf * OBUF + lt * N : obuf * OBUF + lt * N + 512],
                        in_=psum[:, lt * 2 * 512 : (lt * 2 + 1) * 512],
                    ).then_inc(s_cpv, 1)

    return nc


_NC = None


def _get_program():
    global _NC
    if _NC is None:
        _NC = build_program()
    return _NC


def make_in_maps(x, category_ids, weight, bias=None):
    x = np.asarray(x, dtype=np.float32)
    cids = np.asarray(category_ids).astype(np.int64)
    weight = np.asarray(weight, dtype=np.float32)

    # xt: [32, K, L] -> p-major per-k [32, 128, KT, L]
    xt = np.ascontiguousarray(x.transpose(0, 2, 1)).astype(NP_DT)
    xt = xt.reshape(32, KT, 128, L).transpose(0, 2, 1, 3)
    # w: [32, K, N] -> p-major per-k [32, 128, KT, N]
    wg = weight[cids].astype(NP_DT)
    wg = wg.reshape(32, KT, 128, N).transpose(0, 2, 1, 3)
    # pack [xt_k | w_k] chunks: [32, 128, KT, CH] -> [32, 128, BBUF]
    packed = np.concatenate([xt, wg], axis=3).reshape(32, 128, BBUF)

    in_maps = []
    for c in range(8):
        sl = slice(c * B, (c + 1) * B)
        in_maps.append({"inp": np.ascontiguousarray(packed[sl])})
    return in_maps


def run_on_device(in_maps, **kwargs):
    return run_bass_kernel_spmd(_get_program(), in_maps, list(range(8)), **kwargs)


def kernel(x, category_ids, weight, bias=None):
    in_maps = make_in_maps(x, category_ids, weight)
    res = run_on_device(in_maps)
    out = np.concatenate([res.results[c]["out"] for c in range(8)], axis=0)
    # [32, 128, LT*N] p-major -> [32, L, N]
    out = out.astype(np.float32).reshape(32, 128, LT, N).transpose(0, 2, 1, 3)
    out = out.reshape(32, L, N)
    cids = np.asarray(category_ids).astype(np.int64)
    if bias is None:
        bias = np.zeros((np.asarray(weight).shape[0], N), dtype=np.float32)
    out = out + np.asarray(bias, dtype=np.float32)[cids][:, None, :]
    return np.ascontiguousarray(out.astype(np.float32))
